# revision 6
# baseline (speedup 1.0000x reference)
"""Cox proportional-hazards loss (Breslow ties, sqrt of mean) on 8 trn2 cores.

Math: sort records by descending time; risk set of record i is the prefix.
With e = exp(x), Q_j = global inclusive prefix sum of e, and w_j = number of
events in the tied-time segment ending at j (0 if j is not a segment end):
    loss_sum = sum_j w_j * ln(Q_j)  -  sum_i ev_i * x_i
    loss     = sqrt(loss_sum / N)

Sharding: N is split contiguously across 8 cores (2M records each, 4 tiles of
[128 x 4096]; each partition owns a contiguous 4096-record chunk).  Per tile,
exp accumulates per-chunk sums; a chunk-local f32 prefix scan produces q with
initial=0.  The global offset for each chunk (cross-partition lift via a
strict-lower-triangular matmul on PE, cross-tile lift via a tiny exclusive
scan of tile totals, cross-core lift via a 32-byte mid-kernel AllReduce) is
injected through the Ln activation's per-partition bias:
ln(Q) = Ln(q_local + bias).  B partials are then sum(w * lnQ) per tile; the
A partial sum(ev * x) is computed on the otherwise-idle PE engine from a
host-compacted (pure gather of x at event positions) stream.

The host does layout/ordering/integer work only (argsort, gather, segment
detection, per-segment event counts, event-position compaction) plus the
final 8-way partial combine; all floating-point math over the data runs on
device.  x is shipped as bf16 (loss error ~1e-5, tolerance 2e-2).
"""

import os
import sys

for _p in ("/opt/trn_rl_repo", "/root/.axon_site/_ro/trn_rl_repo"):
    if os.path.isdir(_p) and _p not in sys.path:
        sys.path.insert(0, _p)

import numpy as np

import concourse.bass as bass
import concourse.tile as tile
from concourse import bacc, mybir
from concourse.bass_utils import run_bass_kernel_spmd

N = 16777216
NC = 8
NLOC = N // NC          # 2097152 records per core
P = 128
F = 4096                # records per partition-chunk
T = NLOC // (P * F)     # 4 tiles per core
XMF = 8704              # compacted-event stream: 17 x 512 columns
MMC = 512               # matmul moving-dim chunk

_DT = mybir.dt
_ACT = mybir.ActivationFunctionType
_ALU = mybir.AluOpType


def _build(repeat=1):
    nc = bacc.Bacc("TRN2", target_bir_lowering=False, debug=False,
                   num_devices=NC)
    xs_in = nc.dram_tensor("xs", [T, P, F], _DT.bfloat16, kind="ExternalInput")
    w_in = nc.dram_tensor("w", [T, P, F], _DT.uint8, kind="ExternalInput")
    xm_in = nc.dram_tensor("xm", [P, XMF], _DT.bfloat16, kind="ExternalInput")
    mgt_in = nc.dram_tensor("mgt", [1, NC], _DT.float32, kind="ExternalInput")
    moh_in = nc.dram_tensor("moh", [1, NC], _DT.float32, kind="ExternalInput")
    ab_out = nc.dram_tensor("ab", [1, 2], _DT.float32, kind="ExternalOutput")

    with tile.TileContext(nc) as tc:
        with (
            tc.tile_pool(name="io", bufs=2) as io,
            tc.tile_pool(name="wk", bufs=2) as wk,
            tc.tile_pool(name="qp", bufs=1) as qp,
            tc.tile_pool(name="sm", bufs=1) as sm,
            tc.tile_pool(name="ps", bufs=1, space="PSUM") as ps,
            tc.tile_pool(name="psa", bufs=1, space="PSUM") as psa,
            tc.tile_pool(name="dram", bufs=2, space="DRAM") as dram,
        ):
            # constants
            ltri = sm.tile([P, P], _DT.float32)
            nc.gpsimd.memset(ltri[:], 1.0)
            # keep value at (partition q, free p) iff p - q > 0: strict lower
            # triangular as lhsT -> exclusive prefix over partitions
            nc.gpsimd.affine_select(
                ltri[:], ltri[:], pattern=[[1, P]], compare_op=_ALU.is_gt,
                fill=0.0, base=0, channel_multiplier=-1)
            ones_row = sm.tile([1, P], _DT.float32)
            nc.gpsimd.memset(ones_row[:], 1.0)
            ones_col = sm.tile([P, 1], _DT.float32)
            nc.gpsimd.memset(ones_col[:], 1.0)
            ones_col16 = sm.tile([P, 1], _DT.bfloat16)
            nc.gpsimd.memset(ones_col16[:], 1.0)
            mgt = sm.tile([1, NC], _DT.float32)
            nc.sync.dma_start(mgt[:], mgt_in.ap())
            moh = sm.tile([1, NC], _DT.float32)
            nc.sync.dma_start(moh[:], moh_in.ap())

            # ---- A partial on PE: sum over compacted event-x stream ----
            xm = sm.tile([P, XMF], _DT.bfloat16)
            nc.sync.dma_start(xm[:], xm_in.ap())
            a_ps = psa.tile([1, MMC], _DT.float32)
            nmm = XMF // MMC
            for c in range(nmm):
                nc.tensor.matmul(a_ps[:], ones_col16[:],
                                 xm[:, c * MMC:(c + 1) * MMC],
                                 start=(c == 0), stop=(c == nmm - 1))
            a_sb = sm.tile([1, MMC], _DT.float32)
            nc.scalar.copy(a_sb[:], a_ps[:])
            a_tot = sm.tile([1, 1], _DT.float32)
            nc.vector.tensor_reduce(a_tot[:], a_sb[:], mybir.AxisListType.X,
                                    _ALU.add)

            # ---- phase 1: exp + chunk-local scans (q retained in SBUF) ----
            s_p = sm.tile([P, T], _DT.float32)
            qall = qp.tile([P, T * F], _DT.float32)
            for t in range(T * repeat):
                t = t % T
                xs = io.tile([P, F], _DT.bfloat16)
                nc.sync.dma_start(xs[:], xs_in.ap()[t])
                e = wk.tile([P, F], _DT.float32)
                nc.scalar.activation(e[:], xs[:], _ACT.Exp,
                                     accum_out=s_p[:, t:t + 1])
                nc.vector.tensor_tensor_scan(
                    qall[:, t * F:(t + 1) * F], e[:], e[:], 0.0, _ALU.add,
                    _ALU.bypass)

            # ---- cross-tile + cross-core lift ----
            # small matmul outputs share one PSUM bank via column slices
            smalls = ps.tile([1, T + NC + 1], _DT.float32)
            # tile totals [1, T]
            tot_ps = smalls[:, 0:T]
            nc.tensor.matmul(tot_ps, ones_col[:], s_p[:], start=True,
                             stop=True)
            tot_sb = sm.tile([1, T], _DT.float32)
            nc.scalar.copy(tot_sb[:], tot_ps)
            my_tot = sm.tile([1, 1], _DT.float32)
            nc.vector.tensor_reduce(my_tot[:], tot_sb[:],
                                    mybir.AxisListType.X, _ALU.add)

            # collective: v = my_tot * maskgt; AllReduce-add; offs = dot(onehot)
            v_ps = smalls[:, T:T + NC]
            nc.tensor.matmul(v_ps, my_tot[:], mgt[:], start=True, stop=True)
            v_sb = sm.tile([1, NC], _DT.float32)
            nc.scalar.copy(v_sb[:], v_ps)
            cin = dram.tile([1, NC], _DT.float32)
            cout = dram.tile([1, NC], _DT.float32)
            nc.gpsimd.dma_start(cin[:], v_sb[:])
            nc.gpsimd.collective_compute(
                "AllReduce", _ALU.add,
                replica_groups=[list(range(NC))],
                ins=[cin[:].opt()], outs=[cout[:].opt()])
            allv = sm.tile([1, NC], _DT.float32)
            nc.gpsimd.dma_start(allv[:], cout[:])
            junk8 = sm.tile([1, NC], _DT.float32)
            offs = sm.tile([1, 1], _DT.float32)
            nc.vector.scalar_tensor_tensor(
                junk8[:], moh[:], 0.0, allv[:], _ALU.bypass, _ALU.mult,
                accum_out=offs[:])

            # exclusive tile offsets (incl. core offset): scan then subtract
            incl = sm.tile([1, T], _DT.float32)
            nc.vector.tensor_tensor_scan(
                incl[:], tot_sb[:], tot_sb[:], offs[:], _ALU.add, _ALU.bypass)
            excl = sm.tile([1, T], _DT.float32)
            nc.vector.tensor_tensor(excl[:], incl[:], tot_sb[:],
                                    _ALU.subtract)

            # bias[p, t] = sum_{p'<p} s_p[p', t] + excl[t]
            bias_ps = ps.tile([P, T], _DT.float32)
            nc.tensor.matmul(bias_ps[:], ltri[:], s_p[:], start=True,
                             stop=False)
            nc.tensor.matmul(bias_ps[:], ones_row[:], excl[:], start=False,
                             stop=True)
            bias = sm.tile([P, T], _DT.float32)
            nc.scalar.copy(bias[:], bias_ps[:])

            # ---- phase 2: ln(q + bias), B partial = sum(w * lnQ) ----
            acc_b = sm.tile([P, T], _DT.float32)
            for t in range(T * repeat):
                t = t % T
                w = io.tile([P, F], _DT.uint8)
                nc.sync.dma_start(w[:], w_in.ap()[t])
                lnq = wk.tile([P, F], _DT.bfloat16)
                nc.scalar.activation(lnq[:], qall[:, t * F:(t + 1) * F],
                                     _ACT.Ln, bias=bias[:, t:t + 1])
                junk = wk.tile([P, F], _DT.bfloat16)
                nc.vector.scalar_tensor_tensor(
                    junk[:], w[:], 0.0, lnq[:], _ALU.bypass, _ALU.mult,
                    accum_out=acc_b[:, t:t + 1])

            # ---- final: ab = [A_partial, B_partial] ----
            b_p = sm.tile([P, 1], _DT.float32)
            nc.vector.tensor_reduce(b_p[:], acc_b[:], mybir.AxisListType.X,
                                    _ALU.add)
            b_ps = smalls[:, T + NC:T + NC + 1]
            nc.tensor.matmul(b_ps, b_p[:], ones_col[:], start=True,
                             stop=True)
            ab = sm.tile([1, 2], _DT.float32)
            nc.scalar.copy(ab[:, 0:1], a_tot[:])
            nc.scalar.copy(ab[:, 1:2], b_ps)
            nc.sync.dma_start(ab_out.ap(), ab[:])
    nc.compile()
    return nc


_CACHE = {}


def _get(name, builder):
    if name not in _CACHE:
        _CACHE[name] = builder()
    return _CACHE[name]


def _prepare(x, times, events):
    import ml_dtypes

    x = np.asarray(x, dtype=np.float32)
    times = np.asarray(times, dtype=np.int32)
    events = np.asarray(events, dtype=np.int32)
    assert x.shape == (N,)

    order = np.argsort(-times)           # descending time; tie order irrelevant
    xs = x[order].astype(ml_dtypes.bfloat16)
    ts = times[order]
    ev = events[order].astype(np.uint8)

    # segment ends: last index of each run of equal times
    is_end = np.empty(N, dtype=bool)
    np.not_equal(ts[:-1], ts[1:], out=is_end[:-1])
    is_end[-1] = True
    ends = np.flatnonzero(is_end)
    starts = np.empty_like(ends)
    starts[0] = 0
    starts[1:] = ends[:-1] + 1
    seg_ev = np.add.reduceat(ev.astype(np.int64), starts)
    assert seg_ev.max() < 256
    w = np.zeros(N, dtype=np.uint8)
    w[ends] = seg_ev.astype(np.uint8)

    # compacted event-x stream per core (zero padded)
    ev_pos = np.flatnonzero(ev)

    per_core = []
    for c in range(NC):
        cs = c * NLOC
        pos = ev_pos[(ev_pos >= cs) & (ev_pos < cs + NLOC)]
        assert len(pos) <= P * XMF, f"XMF too small: {len(pos)}"
        xm = np.zeros(P * XMF, dtype=ml_dtypes.bfloat16)
        xm[:len(pos)] = xs[pos]
        mgt = np.zeros((1, NC), dtype=np.float32)
        mgt[0, c + 1:] = 1.0
        moh = np.zeros((1, NC), dtype=np.float32)
        moh[0, c] = 1.0
        per_core.append({
            "xs": np.ascontiguousarray(xs[cs:cs + NLOC]).reshape(T, P, F),
            "w": np.ascontiguousarray(w[cs:cs + NLOC]).reshape(T, P, F),
            "xm": xm.reshape(P, XMF),
            "mgt": mgt,
            "moh": moh,
        })
    return per_core


LAST_EXEC_NS = {}


def kernel(x, times, events):
    per_core = _prepare(x, times, events)
    core_ids = list(range(NC))
    trace = bool(int(os.environ.get("BASS_COX_TRACE", "0")))

    nc = _get("main", _build)
    res = run_bass_kernel_spmd(nc, per_core, core_ids=core_ids, trace=trace)
    LAST_EXEC_NS.clear()
    LAST_EXEC_NS["main"] = res.exec_time_ns

    a_tot = 0.0
    b_tot = 0.0
    for c in range(NC):
        ab = res.results[c]["ab"]
        a_tot += float(ab[0, 0])
        b_tot += float(ab[0, 1])
    loss = np.sqrt((b_tot - a_tot) / N)
    return np.float32(loss)


# revision 12
# speedup vs baseline: 1.7641x; 1.7641x over previous
"""Cox proportional-hazards loss (Breslow ties, sqrt of mean) on 8 trn2 cores.

Math: sort records by descending time; risk set of record i is the prefix.
With e = exp(x), Q_j = global inclusive prefix sum of e, and w_j = number of
events in the tied-time segment ending at j (0 if j is not a segment end):
    loss_sum = sum_j w_j * ln(Q_j)  -  sum_i ev_i * x_i
    loss     = sqrt(loss_sum / N)

Because Q is only ever read at tied-time segment ends, and within a segment
the order of records is arbitrary, records are packed on the host into
SW=8-wide "pieces" (segments padded to piece boundaries with -88, whose exp
is 0): piece sums of exp(x) preserve every segment-end prefix while cutting
the scan/ln/weight work by ~7x.  The piece sums are computed on the
otherwise-idle PE engine via block-diagonal matmuls that reduce 8
partition-adjacent slots per piece.

Two launches per core (no mid-kernel collective -- a cross-core sync
inherits variable launch skew, measured at 17-90us):
  pass P: exp(xp) on Act -> piece sums on PE -> writes piece-sum array
          [P, FP] f32, per-partition group sums, exp-total and
          sum(ev*x) (from a host-compacted fp8 event stream, PE matmuls).
  host:   8-way exclusive cumsum of the exp totals (the only cross-core
          dependency; a few scalar adds).
  pass Q: per-group f32 prefix scans of the piece sums; the global offset
          (cross-partition lift via strict-lower-triangular matmul,
          cross-group lift via a tiny exclusive scan, cross-core offset as
          an input) enters via the Ln activation's per-partition bias:
          lnQ = Ln(q_local + bias).  B partial = sum(w * lnQ).

The host does layout/ordering/integer work only (argsort, segment
detection, per-segment event counts, piece packing, event compaction) plus
the 8-way scalar combines; all floating-point math over the data runs on
device.  x ships as fp8(e4m3) (loss rel err ~1e-4, tolerance 2e-2).
"""

import os
import sys

for _p in ("/opt/trn_rl_repo", "/root/.axon_site/_ro/trn_rl_repo"):
    if os.path.isdir(_p) and _p not in sys.path:
        sys.path.insert(0, _p)

import numpy as np

import concourse.bass as bass
import concourse.tile as tile
from concourse import bacc, mybir
from concourse.bass_utils import run_bass_kernel_spmd

N = 16777216
NC = 8
P = 128
SW = 8                  # slots per piece
PB = P // SW            # piece rows per quadrant matmul (16)
MMC = 512               # piece columns per production group
G = 5                   # production groups
FP = G * MMC            # piece columns per partition (2560)
PPC = P * FP            # piece capacity per core (327680)
SF = SW * MMC           # slot columns per group tile (4096)
XMF = 8704              # compacted-event stream columns (17 x 512)
PAD = -88.0             # exp(PAD) == 0 in fp32

_DT = mybir.dt
_ACT = mybir.ActivationFunctionType
_ALU = mybir.AluOpType


def _build_p():
    nc = bacc.Bacc("TRN2", target_bir_lowering=False, debug=False,
                   num_devices=NC)
    xp_in = nc.dram_tensor("xp", [G, P, SF], _DT.float8e4,
                           kind="ExternalInput")
    xm_in = nc.dram_tensor("xm", [P, XMF], _DT.float8e4,
                           kind="ExternalInput")
    epc_out = nc.dram_tensor("epc", [G, P, MMC], _DT.float32,
                             kind="ExternalOutput")
    spg_out = nc.dram_tensor("spg", [P, G], _DT.float32,
                             kind="ExternalOutput")
    stat_out = nc.dram_tensor("stat", [1, 2], _DT.float32,
                              kind="ExternalOutput")

    with tile.TileContext(nc) as tc:
        with (
            tc.tile_pool(name="io", bufs=2) as io,
            tc.tile_pool(name="wk", bufs=2) as wk,
            tc.tile_pool(name="sm", bufs=1) as sm,
            tc.tile_pool(name="pp", bufs=2, space="PSUM") as pp,
            tc.tile_pool(name="psa", bufs=1, space="PSUM") as psa,
        ):
            # B_v[k, m] = 1 iff m == PB*v + k//SW (block-diagonal reducers)
            bmats = []
            for v in range(SW):
                bm = sm.tile([P, P], _DT.bfloat16, name=f"bm{v}")
                nc.gpsimd.memset(bm[:], 1.0)
                # keep iff 0 >= SW*m - P*v - k >= -(SW-1)
                nc.gpsimd.affine_select(
                    bm[:], bm[:], pattern=[[-SW, P]], compare_op=_ALU.is_ge,
                    fill=0.0, base=P * v, channel_multiplier=1)
                nc.gpsimd.affine_select(
                    bm[:], bm[:], pattern=[[SW, P]], compare_op=_ALU.is_ge,
                    fill=0.0, base=-P * v + (SW - 1), channel_multiplier=-1)
                bmats.append(bm)
            ones_col = sm.tile([P, 1], _DT.float32)
            nc.gpsimd.memset(ones_col[:], 1.0)
            ones_col8 = sm.tile([P, 1], _DT.float8e4)
            nc.gpsimd.memset(ones_col8[:], 1.0)

            xm = sm.tile([P, XMF], _DT.float8e4)
            nc.sync.dma_start(xm[:], xm_in.ap())

            s_pg = sm.tile([P, G], _DT.float32)
            for g in range(G):
                xpt = io.tile([P, SF], _DT.float8e4)
                nc.sync.dma_start(xpt[:], xp_in.ap()[g])
                eg = wk.tile([P, SF], _DT.bfloat16)
                nc.scalar.activation(eg[:], xpt[:], _ACT.Exp)
                pp_ps = pp.tile([P, MMC], _DT.float32)
                for v in range(SW):
                    nc.tensor.matmul(pp_ps[:], bmats[v][:],
                                     eg[:, v * MMC:(v + 1) * MMC],
                                     start=(v == 0), stop=(v == SW - 1))
                epg = wk.tile([P, MMC], _DT.float32)
                nc.vector.tensor_scalar_add(epg[:], pp_ps[:], 0.0)
                nc.sync.dma_start(epc_out.ap()[g], epg[:])
                nc.vector.tensor_reduce(s_pg[:, g:g + 1], pp_ps[:],
                                        mybir.AxisListType.X, _ALU.add)
            nc.sync.dma_start(spg_out.ap(), s_pg[:])

            # core exp total
            s_p = sm.tile([P, 1], _DT.float32)
            nc.vector.tensor_reduce(s_p[:], s_pg[:], mybir.AxisListType.X,
                                    _ALU.add)
            tot_ps = psa.tile([1, MMC], _DT.float32)
            nc.tensor.matmul(tot_ps[:, 0:1], s_p[:], ones_col[:], start=True,
                             stop=True)
            stat = sm.tile([1, 2], _DT.float32)
            nc.scalar.copy(stat[:, 1:2], tot_ps[:, 0:1])

            # A partial: sum of compacted event-x stream (PE)
            a_ps = psa.tile([1, MMC], _DT.float32, name="aps")
            nmm = XMF // MMC
            for c in range(nmm):
                nc.tensor.matmul(a_ps[:], ones_col8[:],
                                 xm[:, c * MMC:(c + 1) * MMC],
                                 start=(c == 0), stop=(c == nmm - 1))
            a_sb = sm.tile([1, MMC], _DT.float32)
            nc.scalar.copy(a_sb[:], a_ps[:])
            nc.vector.tensor_reduce(stat[:, 0:1], a_sb[:],
                                    mybir.AxisListType.X, _ALU.add)
            nc.sync.dma_start(stat_out.ap(), stat[:])
    nc.compile()
    return nc


def _build_q():
    nc = bacc.Bacc("TRN2", target_bir_lowering=False, debug=False,
                   num_devices=NC)
    epc_in = nc.dram_tensor("epc", [P, G * MMC], _DT.float32,
                            kind="ExternalInput")
    spg_in = nc.dram_tensor("spg", [P, G], _DT.float32, kind="ExternalInput")
    w_in = nc.dram_tensor("w", [P, G * MMC], _DT.uint8, kind="ExternalInput")
    off_in = nc.dram_tensor("off", [1, 1], _DT.float32, kind="ExternalInput")
    b_out = nc.dram_tensor("b", [1, 1], _DT.float32, kind="ExternalOutput")

    with tile.TileContext(nc) as tc:
        with (
            tc.tile_pool(name="io", bufs=2) as io,
            tc.tile_pool(name="wk", bufs=2) as wk,
            tc.tile_pool(name="sm", bufs=1) as sm,
            tc.tile_pool(name="ps", bufs=1, space="PSUM") as ps,
        ):
            epc = sm.tile([P, G * MMC], _DT.float32)
            nc.sync.dma_start(epc[:], epc_in.ap())
            w = sm.tile([P, G * MMC], _DT.uint8)
            nc.sync.dma_start(w[:], w_in.ap())
            s_pg = sm.tile([P, G], _DT.float32)
            nc.sync.dma_start(s_pg[:], spg_in.ap())
            off = sm.tile([1, 1], _DT.float32)
            nc.sync.dma_start(off[:], off_in.ap())

            ltri = sm.tile([P, P], _DT.float32)
            nc.gpsimd.memset(ltri[:], 1.0)
            nc.gpsimd.affine_select(
                ltri[:], ltri[:], pattern=[[1, P]], compare_op=_ALU.is_gt,
                fill=0.0, base=0, channel_multiplier=-1)
            ones_row = sm.tile([1, P], _DT.float32)
            nc.gpsimd.memset(ones_row[:], 1.0)
            ones_col = sm.tile([P, 1], _DT.float32)
            nc.gpsimd.memset(ones_col[:], 1.0)

            # chunk-local scans
            qpc = sm.tile([P, G * MMC], _DT.float32, name="qpc")
            for g in range(G):
                sl = slice(g * MMC, (g + 1) * MMC)
                nc.vector.tensor_tensor_scan(
                    qpc[:, sl], epc[:, sl], epc[:, sl], 0.0, _ALU.add,
                    _ALU.bypass)

            # lifts: group totals, exclusive group offsets (+ core offset)
            smalls = ps.tile([1, G + 1], _DT.float32)
            tot_ps = smalls[:, 0:G]
            nc.tensor.matmul(tot_ps, ones_col[:], s_pg[:], start=True,
                             stop=True)
            tot_sb = sm.tile([1, G], _DT.float32)
            nc.scalar.copy(tot_sb[:], tot_ps)
            incl = sm.tile([1, G], _DT.float32)
            nc.vector.tensor_tensor_scan(
                incl[:], tot_sb[:], tot_sb[:], off[:], _ALU.add, _ALU.bypass)
            excl = sm.tile([1, G], _DT.float32)
            nc.vector.tensor_tensor(excl[:], incl[:], tot_sb[:],
                                    _ALU.subtract)

            # bias[p, g] = sum_{p'<p} s_pg[p', g] + excl[g]
            bias_ps = ps.tile([P, G], _DT.float32)
            nc.tensor.matmul(bias_ps[:], ltri[:], s_pg[:], start=True,
                             stop=False)
            nc.tensor.matmul(bias_ps[:], ones_row[:], excl[:], start=False,
                             stop=True)
            bias = sm.tile([P, G], _DT.float32)
            nc.scalar.copy(bias[:], bias_ps[:])

            # ln(q + bias), B partial = sum(w * lnQ)
            acc_b = sm.tile([P, G], _DT.float32)
            for g in range(G):
                sl = slice(g * MMC, (g + 1) * MMC)
                lnq = wk.tile([P, MMC], _DT.bfloat16)
                nc.scalar.activation(lnq[:], qpc[:, sl], _ACT.Ln,
                                     bias=bias[:, g:g + 1])
                junk = wk.tile([P, MMC], _DT.bfloat16)
                nc.vector.scalar_tensor_tensor(
                    junk[:], w[:, sl], 0.0, lnq[:], _ALU.bypass, _ALU.mult,
                    accum_out=acc_b[:, g:g + 1])

            b_p = sm.tile([P, 1], _DT.float32)
            nc.vector.tensor_reduce(b_p[:], acc_b[:], mybir.AxisListType.X,
                                    _ALU.add)
            b_ps = smalls[:, G:G + 1]
            nc.tensor.matmul(b_ps, b_p[:], ones_col[:], start=True, stop=True)
            b_sb = sm.tile([1, 1], _DT.float32)
            nc.scalar.copy(b_sb[:], b_ps)
            nc.sync.dma_start(b_out.ap(), b_sb[:])
    nc.compile()
    return nc


_CACHE = {}


def _get(name, builder):
    if name not in _CACHE:
        _CACHE[name] = builder()
    return _CACHE[name]


def _prepare(x, times, events):
    import ml_dtypes

    f8 = ml_dtypes.float8_e4m3fn
    x = np.asarray(x, dtype=np.float32)
    times = np.asarray(times, dtype=np.int32)
    events = np.asarray(events, dtype=np.int32)
    assert x.shape == (N,)

    order = np.argsort(-times)           # descending time; tie order irrelevant
    xs = x[order]
    ts = times[order]
    ev = events[order].astype(bool)

    # segments = runs of equal times
    is_end = np.empty(N, dtype=bool)
    np.not_equal(ts[:-1], ts[1:], out=is_end[:-1])
    is_end[-1] = True
    ends = np.flatnonzero(is_end)
    starts = np.empty_like(ends)
    starts[0] = 0
    starts[1:] = ends[:-1] + 1
    seg_len = np.diff(np.append(starts, N))
    seg_ev = np.add.reduceat(ev.astype(np.int64), starts)
    assert seg_ev.max() < 256

    is_start = np.empty(N, dtype=bool)
    is_start[0] = True
    is_start[1:] = is_end[:-1]
    seg_id = np.cumsum(is_start) - 1
    off_in_seg = np.arange(N, dtype=np.int64) - starts[seg_id]

    pieces_per_seg = (seg_len + SW - 1) // SW
    piece_base = np.concatenate([[0], np.cumsum(pieces_per_seg)[:-1]])
    n_pieces = int(piece_base[-1] + pieces_per_seg[-1])
    assert n_pieces <= NC * PPC, (n_pieces, NC * PPC)
    per_core = -(-n_pieces // NC)

    l = piece_base[seg_id] + off_in_seg // SW
    slot = off_in_seg % SW
    c = l // per_core
    lp = l % per_core
    p = lp // FP
    f = lp % FP
    g = f // MMC
    n = f % MMC
    v = p // PB
    k = SW * (p % PB) + slot

    # xp[c, g, k, v*MMC + n] = xs
    xp = np.full(NC * G * P * SF, PAD, dtype=f8)
    dest = ((c * G + g) * P + k) * SF + v * MMC + n
    xp[dest] = xs.astype(f8)
    xp = xp.reshape(NC, G, P, SF)

    # w over pieces
    w = np.zeros(NC * PPC, dtype=np.uint8)
    last_piece = piece_base + pieces_per_seg - 1
    w[(last_piece // per_core) * PPC + last_piece % per_core] = seg_ev
    w = w.reshape(NC, P, FP)

    # compacted event-x stream, assigned to the core owning the record
    ev_pos = np.flatnonzero(ev)
    ev_core = c[ev_pos]
    xm = np.zeros((NC, P * XMF), dtype=f8)
    for cc in range(NC):
        vals = xs[ev_pos[ev_core == cc]]
        assert len(vals) <= P * XMF, (cc, len(vals))
        xm[cc, :len(vals)] = vals.astype(f8)
    xm = xm.reshape(NC, P, XMF)

    in_p = [{"xp": xp[cc], "xm": xm[cc]} for cc in range(NC)]
    w_per_core = [w[cc] for cc in range(NC)]
    return in_p, w_per_core


LAST_EXEC_NS = {}


def kernel(x, times, events):
    in_p, w_per_core = _prepare(x, times, events)
    core_ids = list(range(NC))
    trace = bool(int(os.environ.get("BASS_COX_TRACE", "0")))

    nc_p = _get("p", _build_p)
    res_p = run_bass_kernel_spmd(nc_p, in_p, core_ids=core_ids, trace=trace)

    tots = np.array([res_p.results[cc]["stat"][0, 1] for cc in range(NC)],
                    dtype=np.float64)
    offs = np.cumsum(tots) - tots
    a_tot = float(sum(res_p.results[cc]["stat"][0, 0] for cc in range(NC)))

    nc_q = _get("q", _build_q)
    in_q = []
    for cc in range(NC):
        in_q.append({
            "epc": np.ascontiguousarray(
                res_p.results[cc]["epc"].transpose(1, 0, 2)).reshape(
                    P, G * MMC),
            "spg": res_p.results[cc]["spg"],
            "w": w_per_core[cc].reshape(P, G * MMC),
            "off": np.array([[offs[cc]]], dtype=np.float32),
        })
    res_q = run_bass_kernel_spmd(nc_q, in_q, core_ids=core_ids, trace=trace)

    LAST_EXEC_NS.clear()
    LAST_EXEC_NS["p"] = res_p.exec_time_ns
    LAST_EXEC_NS["q"] = res_q.exec_time_ns

    b_tot = float(sum(res_q.results[cc]["b"][0, 0] for cc in range(NC)))
    loss = np.sqrt((b_tot - a_tot) / N)
    return np.float32(loss)


# revision 18
# speedup vs baseline: 2.0585x; 1.1669x over previous
"""Cox proportional-hazards loss (Breslow ties, sqrt of mean) on 8 trn2 cores.

Math: sort records by descending time; risk set of record i is the prefix.
With e = exp(x), Q_j = global inclusive prefix sum of e, and w_j = number of
events in the tied-time segment ending at j (0 if j is not a segment end):
    loss_sum = sum_j w_j * ln(Q_j)  -  sum_i ev_i * x_i
    loss     = sqrt(loss_sum / N)

Because Q is only ever read at tied-time segment ends, and within a segment
the order of records is arbitrary, records are packed on the host into
SW=8-wide "pieces" (segments padded to piece boundaries with -88, whose exp
is 0): piece sums of exp(x) preserve every segment-end prefix while cutting
the scan/ln/weight work by ~7x.  The piece sums are computed on the
otherwise-idle PE engine via block-diagonal matmuls that reduce 8
partition-adjacent slots per piece.

Two launches per core (no mid-kernel collective -- a cross-core sync
inherits variable launch skew, measured at 17-90us):
  pass P: exp(xp) on Act -> piece sums on PE -> writes piece-sum array
          [P, FP] f32, per-partition group sums, exp-total and
          sum(ev*x) (from a host-compacted fp8 event stream, PE matmuls).
  host:   8-way exclusive cumsum of the exp totals (the only cross-core
          dependency; a few scalar adds).
  pass Q: per-group f32 prefix scans of the piece sums; the global offset
          (cross-partition lift via strict-lower-triangular matmul,
          cross-group lift via a tiny exclusive scan, cross-core offset as
          an input) enters via the Ln activation's per-partition bias:
          lnQ = Ln(q_local + bias).  B partial = sum(w * lnQ).

The host does layout/ordering/integer work only (argsort, segment
detection, per-segment event counts, piece packing, event compaction) plus
the 8-way scalar combines; all floating-point math over the data runs on
device.  x ships as fp8(e4m3) (loss rel err ~1e-4, tolerance 2e-2).
"""

import os
import sys

for _p in ("/opt/trn_rl_repo", "/root/.axon_site/_ro/trn_rl_repo"):
    if os.path.isdir(_p) and _p not in sys.path:
        sys.path.insert(0, _p)

import numpy as np

import concourse.bass as bass
import concourse.tile as tile
from concourse import bacc, mybir
from concourse.bass_utils import run_bass_kernel_spmd

N = 16777216
NC = 8
P = 128
SW = 8                  # slots per piece
PB = P // SW            # piece rows per quadrant matmul (16)
MMC = 512               # piece columns per production group
G = 5                   # production groups
FP = G * MMC            # piece columns per partition (2560)
PPC = P * FP            # piece capacity per core (327680)
SF = SW * MMC           # slot columns per group tile (4096)
XMF = 8704              # compacted-event stream columns (17 x 512)
PAD = -88.0             # exp(PAD) == 0 in fp32

_DT = mybir.dt
_ACT = mybir.ActivationFunctionType
_ALU = mybir.AluOpType


def _build_p():
    nc = bacc.Bacc("TRN2", target_bir_lowering=False, debug=False,
                   num_devices=NC)
    xp_in = nc.dram_tensor("xp", [G, P, SF], _DT.float8e4,
                           kind="ExternalInput")
    xm_in = nc.dram_tensor("xm", [P, XMF], _DT.float8e4,
                           kind="ExternalInput")
    epc_out = nc.dram_tensor("epc", [G, P, MMC], _DT.float32,
                             kind="ExternalOutput")
    spg_out = nc.dram_tensor("spg", [P, G], _DT.float32,
                             kind="ExternalOutput")
    stat_out = nc.dram_tensor("stat", [1, 2], _DT.float32,
                              kind="ExternalOutput")

    with tile.TileContext(nc) as tc:
        with (
            tc.tile_pool(name="io", bufs=2) as io,
            tc.tile_pool(name="wk", bufs=2) as wk,
            tc.tile_pool(name="sm", bufs=1) as sm,
            tc.tile_pool(name="pp", bufs=2, space="PSUM") as pp,
            tc.tile_pool(name="psa", bufs=1, space="PSUM") as psa,
        ):
            # B_v[k, m] = 1 iff m == PB*v + k//SW (block-diagonal reducers)
            bmats = []
            for v in range(SW):
                bm = sm.tile([P, P], _DT.bfloat16, name=f"bm{v}")
                nc.gpsimd.memset(bm[:], 1.0)
                # keep iff 0 >= SW*m - P*v - k >= -(SW-1)
                nc.gpsimd.affine_select(
                    bm[:], bm[:], pattern=[[-SW, P]], compare_op=_ALU.is_ge,
                    fill=0.0, base=P * v, channel_multiplier=1)
                nc.gpsimd.affine_select(
                    bm[:], bm[:], pattern=[[SW, P]], compare_op=_ALU.is_ge,
                    fill=0.0, base=-P * v + (SW - 1), channel_multiplier=-1)
                bmats.append(bm)
            ones_col = sm.tile([P, 1], _DT.float32)
            nc.gpsimd.memset(ones_col[:], 1.0)
            ones_col8 = sm.tile([P, 1], _DT.float8e4)
            nc.gpsimd.memset(ones_col8[:], 1.0)

            s_pg = sm.tile([P, G], _DT.float32)
            xm = sm.tile([P, XMF], _DT.float8e4)
            for g in range(G):
                xpt = io.tile([P, SF], _DT.float8e4)
                nc.sync.dma_start(xpt[:], xp_in.ap()[g])
                if g == G - 1:
                    # issue after the xp requests so it doesn't delay them
                    nc.sync.dma_start(xm[:], xm_in.ap())
                eg = wk.tile([P, SF], _DT.bfloat16)
                nc.scalar.activation(eg[:], xpt[:], _ACT.Exp)
                pp_ps = pp.tile([P, MMC], _DT.float32)
                for v in range(SW):
                    nc.tensor.matmul(pp_ps[:], bmats[v][:],
                                     eg[:, v * MMC:(v + 1) * MMC],
                                     start=(v == 0), stop=(v == SW - 1))
                epg = wk.tile([P, MMC], _DT.float32)
                nc.vector.tensor_scalar(epg[:], pp_ps[:], 0.0, 0.0,
                                        _ALU.add, _ALU.add,
                                        accum_out=s_pg[:, g:g + 1])
                nc.sync.dma_start(epc_out.ap()[g], epg[:])
            nc.sync.dma_start(spg_out.ap(), s_pg[:])

            # core exp total
            s_p = sm.tile([P, 1], _DT.float32)
            nc.vector.tensor_reduce(s_p[:], s_pg[:], mybir.AxisListType.X,
                                    _ALU.add)
            tot_ps = psa.tile([1, MMC], _DT.float32)
            nc.tensor.matmul(tot_ps[:, 0:1], s_p[:], ones_col[:], start=True,
                             stop=True)
            stat = sm.tile([1, 2], _DT.float32)
            nc.scalar.copy(stat[:, 1:2], tot_ps[:, 0:1])

            # A partial: sum of compacted event-x stream (PE)
            a_ps = psa.tile([1, MMC], _DT.float32, name="aps")
            nmm = XMF // MMC
            for c in range(nmm):
                nc.tensor.matmul(a_ps[:], ones_col8[:],
                                 xm[:, c * MMC:(c + 1) * MMC],
                                 start=(c == 0), stop=(c == nmm - 1))
            a_sb = sm.tile([1, MMC], _DT.float32)
            nc.scalar.copy(a_sb[:], a_ps[:])
            nc.vector.tensor_reduce(stat[:, 0:1], a_sb[:],
                                    mybir.AxisListType.X, _ALU.add)
            nc.sync.dma_start(stat_out.ap(), stat[:])
    nc.compile()
    return nc


def _build_q():
    nc = bacc.Bacc("TRN2", target_bir_lowering=False, debug=False,
                   num_devices=NC)
    epc_in = nc.dram_tensor("epc", [G, P, MMC], _DT.float32,
                            kind="ExternalInput")
    spg_in = nc.dram_tensor("spg", [P, G], _DT.float32, kind="ExternalInput")
    w_in = nc.dram_tensor("w", [P, G * MMC], _DT.uint8, kind="ExternalInput")
    off_in = nc.dram_tensor("off", [1, 1], _DT.float32, kind="ExternalInput")
    b_out = nc.dram_tensor("b", [1, 1], _DT.float32, kind="ExternalOutput")

    with tile.TileContext(nc) as tc:
        with (
            tc.tile_pool(name="io", bufs=2) as io,
            tc.tile_pool(name="wk", bufs=2) as wk,
            tc.tile_pool(name="sm", bufs=1) as sm,
            tc.tile_pool(name="ps", bufs=1, space="PSUM") as ps,
        ):
            # inputs: s_pg/off first (the bias chain needs only these),
            # then per-group piece sums, then weights
            s_pg = sm.tile([P, G], _DT.float32)
            nc.sync.dma_start(s_pg[:], spg_in.ap())
            off = sm.tile([1, 1], _DT.float32)
            nc.sync.dma_start(off[:], off_in.ap())
            epc = sm.tile([P, G * MMC], _DT.float32)
            for g in range(G):
                nc.sync.dma_start(epc[:, g * MMC:(g + 1) * MMC],
                                  epc_in.ap()[g])
            w = sm.tile([P, G * MMC], _DT.uint8)
            nc.sync.dma_start(w[:], w_in.ap())

            ltri = sm.tile([P, P], _DT.float32)
            nc.gpsimd.memset(ltri[:], 1.0)
            nc.gpsimd.affine_select(
                ltri[:], ltri[:], pattern=[[1, P]], compare_op=_ALU.is_gt,
                fill=0.0, base=0, channel_multiplier=-1)
            ones_row = sm.tile([1, P], _DT.float32)
            nc.gpsimd.memset(ones_row[:], 1.0)
            ones_col = sm.tile([P, 1], _DT.float32)
            nc.gpsimd.memset(ones_col[:], 1.0)
            # preload the Ln activation table off the critical path
            dummy = sm.tile([1, 1], _DT.float32)
            nc.gpsimd.memset(dummy[:], 1.0)
            nc.scalar.activation(dummy[:], dummy[:], _ACT.Ln)

            # lifts first: they depend only on inputs, so the Ln/stt pipeline
            # can overlap the scan chain
            smalls = ps.tile([1, G + 1], _DT.float32)
            tot_ps = smalls[:, 0:G]
            nc.tensor.matmul(tot_ps, ones_col[:], s_pg[:], start=True,
                             stop=True)
            tot_sb = sm.tile([1, G], _DT.float32)
            nc.scalar.copy(tot_sb[:], tot_ps)
            incl = sm.tile([1, G], _DT.float32)
            nc.vector.tensor_tensor_scan(
                incl[:], tot_sb[:], tot_sb[:], off[:], _ALU.add, _ALU.bypass)
            excl = sm.tile([1, G], _DT.float32)
            nc.vector.tensor_tensor(excl[:], incl[:], tot_sb[:],
                                    _ALU.subtract)

            # bias[p, g] = sum_{p'<p} s_pg[p', g] + excl[g]
            bias_ps = ps.tile([P, G], _DT.float32)
            nc.tensor.matmul(bias_ps[:], ltri[:], s_pg[:], start=True,
                             stop=False)
            nc.tensor.matmul(bias_ps[:], ones_row[:], excl[:], start=False,
                             stop=True)
            bias = sm.tile([P, G], _DT.float32)
            nc.scalar.copy(bias[:], bias_ps[:])

            # chunk-local scans on Vector; Ln on Act overlaps the scan chain
            # (bias depends only on inputs); stt for group g is emitted one
            # group late so it never stalls the vector queue waiting on Ln
            qpc = sm.tile([P, G * MMC], _DT.float32, name="qpc")
            acc_b = sm.tile([P, G], _DT.float32)
            lnqs = []

            def _stt(g):
                sl = slice(g * MMC, (g + 1) * MMC)
                junk = wk.tile([P, MMC], _DT.bfloat16, name=f"junk{g}")
                nc.vector.scalar_tensor_tensor(
                    junk[:], w[:, sl], 0.0, lnqs[g][:], _ALU.bypass,
                    _ALU.mult, accum_out=acc_b[:, g:g + 1])

            for g in range(G):
                sl = slice(g * MMC, (g + 1) * MMC)
                nc.vector.tensor_tensor_scan(
                    qpc[:, sl], epc[:, sl], epc[:, sl], 0.0, _ALU.add,
                    _ALU.bypass)
                lnq = wk.tile([P, MMC], _DT.bfloat16, name=f"lnq{g}")
                nc.scalar.activation(lnq[:], qpc[:, sl], _ACT.Ln,
                                     bias=bias[:, g:g + 1])
                lnqs.append(lnq)
                if g >= 1:
                    _stt(g - 1)
            _stt(G - 1)

            b_p = sm.tile([P, 1], _DT.float32)
            nc.vector.tensor_reduce(b_p[:], acc_b[:], mybir.AxisListType.X,
                                    _ALU.add)
            b_ps = smalls[:, G:G + 1]
            nc.tensor.matmul(b_ps, b_p[:], ones_col[:], start=True, stop=True)
            b_sb = sm.tile([1, 1], _DT.float32)
            nc.scalar.copy(b_sb[:], b_ps)
            nc.sync.dma_start(b_out.ap(), b_sb[:])
    nc.compile()
    return nc


_CACHE = {}


def _get(name, builder):
    if name not in _CACHE:
        _CACHE[name] = builder()
    return _CACHE[name]


def _prepare(x, times, events):
    import ml_dtypes

    f8 = ml_dtypes.float8_e4m3fn
    x = np.asarray(x, dtype=np.float32)
    times = np.asarray(times, dtype=np.int32)
    events = np.asarray(events, dtype=np.int32)
    assert x.shape == (N,)

    order = np.argsort(-times)           # descending time; tie order irrelevant
    xs = x[order]
    ts = times[order]
    ev = events[order].astype(bool)

    # segments = runs of equal times
    is_end = np.empty(N, dtype=bool)
    np.not_equal(ts[:-1], ts[1:], out=is_end[:-1])
    is_end[-1] = True
    ends = np.flatnonzero(is_end)
    starts = np.empty_like(ends)
    starts[0] = 0
    starts[1:] = ends[:-1] + 1
    seg_len = np.diff(np.append(starts, N))
    seg_ev = np.add.reduceat(ev.astype(np.int64), starts)
    assert seg_ev.max() < 256

    is_start = np.empty(N, dtype=bool)
    is_start[0] = True
    is_start[1:] = is_end[:-1]
    seg_id = np.cumsum(is_start) - 1
    off_in_seg = np.arange(N, dtype=np.int64) - starts[seg_id]

    pieces_per_seg = (seg_len + SW - 1) // SW
    piece_base = np.concatenate([[0], np.cumsum(pieces_per_seg)[:-1]])
    n_pieces = int(piece_base[-1] + pieces_per_seg[-1])
    assert n_pieces <= NC * PPC, (n_pieces, NC * PPC)
    per_core = -(-n_pieces // NC)

    l = piece_base[seg_id] + off_in_seg // SW
    slot = off_in_seg % SW
    c = l // per_core
    lp = l % per_core
    p = lp // FP
    f = lp % FP
    g = f // MMC
    n = f % MMC
    v = p // PB
    k = SW * (p % PB) + slot

    # xp[c, g, k, v*MMC + n] = xs
    xp = np.full(NC * G * P * SF, PAD, dtype=f8)
    dest = ((c * G + g) * P + k) * SF + v * MMC + n
    xp[dest] = xs.astype(f8)
    xp = xp.reshape(NC, G, P, SF)

    # w over pieces
    w = np.zeros(NC * PPC, dtype=np.uint8)
    last_piece = piece_base + pieces_per_seg - 1
    w[(last_piece // per_core) * PPC + last_piece % per_core] = seg_ev
    w = w.reshape(NC, P, FP)

    # compacted event-x stream, assigned to the core owning the record
    ev_pos = np.flatnonzero(ev)
    ev_core = c[ev_pos]
    xm = np.zeros((NC, P * XMF), dtype=f8)
    for cc in range(NC):
        vals = xs[ev_pos[ev_core == cc]]
        assert len(vals) <= P * XMF, (cc, len(vals))
        xm[cc, :len(vals)] = vals.astype(f8)
    xm = xm.reshape(NC, P, XMF)

    in_p = [{"xp": xp[cc], "xm": xm[cc]} for cc in range(NC)]
    w_per_core = [w[cc] for cc in range(NC)]
    return in_p, w_per_core


LAST_EXEC_NS = {}


def kernel(x, times, events):
    in_p, w_per_core = _prepare(x, times, events)
    core_ids = list(range(NC))
    trace = bool(int(os.environ.get("BASS_COX_TRACE", "0")))

    nc_p = _get("p", _build_p)
    res_p = run_bass_kernel_spmd(nc_p, in_p, core_ids=core_ids, trace=trace)

    tots = np.array([res_p.results[cc]["stat"][0, 1] for cc in range(NC)],
                    dtype=np.float64)
    offs = np.cumsum(tots) - tots
    a_tot = float(sum(res_p.results[cc]["stat"][0, 0] for cc in range(NC)))

    nc_q = _get("q", _build_q)
    in_q = []
    for cc in range(NC):
        in_q.append({
            "epc": np.ascontiguousarray(
                res_p.results[cc]["epc"].transpose(1, 0, 2)).reshape(
                    P, G * MMC),
            "spg": res_p.results[cc]["spg"],
            "w": w_per_core[cc].reshape(P, G * MMC),
            "off": np.array([[offs[cc]]], dtype=np.float32),
        })
    res_q = run_bass_kernel_spmd(nc_q, in_q, core_ids=core_ids, trace=trace)

    LAST_EXEC_NS.clear()
    LAST_EXEC_NS["p"] = res_p.exec_time_ns
    LAST_EXEC_NS["q"] = res_q.exec_time_ns

    b_tot = float(sum(res_q.results[cc]["b"][0, 0] for cc in range(NC)))
    loss = np.sqrt((b_tot - a_tot) / N)
    return np.float32(loss)
